# revision 1
# baseline (speedup 1.0000x reference)
"""Two-layer SAGEConv (mean aggregation) GNN on 8 trn2 NeuronCores.

Strategy (dst-sharded graph parallel):
  - Core c owns dst nodes [5000c, 5000(c+1)). Edges routed to the core that
    owns their dst. Feature tables (x, then h) are replicated to every core
    in a padded row layout [8*5120, 128] bf16.
  - Per core, dst nodes are processed in 40 ranges of 128. Edges of a range
    are packed into 128-edge blocks; messages x[src] are fetched with
    dma_gather (int16 local idx, so the table is split in two 20480-row
    halves) and reduced with PE matmuls against fp8 0/1 one-hot blocks:
        aggT[f, slot] += msg_blk[e, f].T @ onehot_blk[e, slot]
    The per-dst 1/deg scale is applied afterwards along the free dim.
  - Layer output: hT = relu(W_l.T @ meanT + W_r.T @ xT + b). Layer 1
    additionally PE-transposes hT back to row-major to rebuild the h table
    (host assembles the full table between the two launches).
"""
import numpy as np
import ml_dtypes
from contextlib import ExitStack

import concourse.bass as bass
import concourse.mybir as mybir
import concourse.tile as tile
from concourse import bacc
from concourse.library_config import mlp
from concourse import bass_utils

BF16 = mybir.dt.bfloat16
F32 = mybir.dt.float32
FP8 = mybir.dt.float8e4
I16 = mybir.dt.int16
NP_BF16 = ml_dtypes.bfloat16
NP_FP8 = ml_dtypes.float8_e4m3

N = 40000
D = 128
CORES = 8
NPC = 5000            # nodes per core
NPAD = 5120           # padded nodes per core (40 * 128)
RANGES = 40           # dst ranges of 128 nodes per core
TROWS = CORES * NPAD  # 40960 table rows
HALF_ROWS = TROWS // 2  # 20480, fits int16 local indices
RPG = 5               # ranges per gather/onehot group
GROUPS = RANGES // RPG

_prog_cache = {}


def build_program(BPH, layer):
    """One SPMD program for one SAGEConv layer. BPH = blocks per (range, half)."""
    CELL = BPH * 128
    BPR = 2 * BPH                 # blocks per range
    TOT_BLKS = RANGES * BPR
    BPG = RPG * BPR               # blocks per group
    IDX_COLS = RANGES * CELL // 16

    nc = bacc.Bacc("TRN2", target_bir_lowering=False, debug=False)
    table = nc.dram_tensor("table", [TROWS, D], BF16, kind="ExternalInput")
    ohp = nc.dram_tensor("ohp", [128, TOT_BLKS, 128], FP8, kind="ExternalInput")
    idxA_d = nc.dram_tensor("idxA", [128, IDX_COLS], I16, kind="ExternalInput")
    idxB_d = nc.dram_tensor("idxB", [128, IDX_COLS], I16, kind="ExternalInput")
    xT_d = nc.dram_tensor("xT", [128, NPAD], BF16, kind="ExternalInput")
    recip_d = nc.dram_tensor("recipb", [128, NPAD], BF16, kind="ExternalInput")
    Wl_d = nc.dram_tensor("Wl", [128, 128], BF16, kind="ExternalInput")
    Wr_d = nc.dram_tensor("Wr", [128, 128], BF16, kind="ExternalInput")
    b_d = nc.dram_tensor("bvec", [128, 1], F32, kind="ExternalInput")
    if layer == 1:
        ident_d = nc.dram_tensor("ident", [128, 128], BF16, kind="ExternalInput")
        hout = nc.dram_tensor("hout", [NPAD, D], BF16, kind="ExternalOutput")
    else:
        tout = nc.dram_tensor("tout", [128, NPAD], F32, kind="ExternalOutput")

    with tile.TileContext(nc) as tc, ExitStack() as ctx:
        const = ctx.enter_context(tc.tile_pool(name="const", bufs=1))
        pmA = ctx.enter_context(tc.tile_pool(name="msgA", bufs=2))
        pmB = ctx.enter_context(tc.tile_pool(name="msgB", bufs=2))
        poh = ctx.enter_context(tc.tile_pool(name="oh", bufs=2))
        psagg = ctx.enter_context(tc.tile_pool(name="psagg", bufs=3, space="PSUM"))
        pslin = ctx.enter_context(tc.tile_pool(name="pslin", bufs=2, space="PSUM"))
        pmean = ctx.enter_context(tc.tile_pool(name="mean", bufs=3))
        if layer == 1:
            pstr = ctx.enter_context(tc.tile_pool(name="pstr", bufs=2, space="PSUM"))
            phT = ctx.enter_context(tc.tile_pool(name="hT", bufs=3))

        nc.gpsimd.load_library(mlp)

        idxA = const.tile([128, IDX_COLS], I16)
        nc.sync.dma_start(idxA[:], idxA_d[:])
        idxB = const.tile([128, IDX_COLS], I16)
        nc.sync.dma_start(idxB[:], idxB_d[:])
        xT = const.tile([128, NPAD], BF16)
        nc.sync.dma_start(xT[:], xT_d[:])
        recip = const.tile([128, NPAD], BF16)
        nc.sync.dma_start(recip[:], recip_d[:])
        Wl = const.tile([128, 128], BF16)
        nc.sync.dma_start(Wl[:], Wl_d[:])
        Wr = const.tile([128, 128], BF16)
        nc.sync.dma_start(Wr[:], Wr_d[:])
        bv = const.tile([128, 1], F32)
        nc.sync.dma_start(bv[:], b_d[:])
        if layer == 1:
            ident = const.tile([128, 128], BF16)
            nc.sync.dma_start(ident[:], ident_d[:])
            hstage = const.tile([128, RANGES, 128], BF16)
        else:
            ostage = const.tile([128, NPAD], F32)

        tabA = table[0:HALF_ROWS, :]
        tabB = table[HALF_ROWS:TROWS, :]
        GN = RPG * CELL           # gathered idxs per call
        GC = GN // 16             # idx cols per call

        for g in range(GROUPS):
            msgA = pmA.tile([128, RPG * BPH, 128], BF16)
            nc.gpsimd.dma_gather(msgA[:], tabA, idxA[:, g * GC:(g + 1) * GC],
                                 GN, GN, D, single_packet=False)
            msgB = pmB.tile([128, RPG * BPH, 128], BF16)
            nc.gpsimd.dma_gather(msgB[:], tabB, idxB[:, g * GC:(g + 1) * GC],
                                 GN, GN, D, single_packet=False)
            oh = poh.tile([128, BPG, 128], FP8)
            nc.sync.dma_start(oh[:], ohp[:, g * BPG:(g + 1) * BPG, :])

            for rr in range(RPG):
                r = g * RPG + rr
                ps = psagg.tile([128, 128], F32)
                for b in range(BPH):
                    nc.tensor.matmul(ps[:], msgA[:, rr * BPH + b, :],
                                     oh[:, rr * BPR + b, :],
                                     start=(b == 0), stop=False)
                for b in range(BPH):
                    nc.tensor.matmul(ps[:], msgB[:, rr * BPH + b, :],
                                     oh[:, rr * BPR + BPH + b, :],
                                     start=False, stop=(b == BPH - 1))
                mean = pmean.tile([128, 128], BF16)
                nc.vector.tensor_mul(mean[:], ps[:],
                                     recip[:, r * 128:(r + 1) * 128])
                ps2 = pslin.tile([128, 128], F32)
                nc.tensor.matmul(ps2[:], Wl[:], mean[:], start=True, stop=False)
                nc.tensor.matmul(ps2[:], Wr[:], xT[:, r * 128:(r + 1) * 128],
                                 start=False, stop=True)
                if layer == 1:
                    hT = phT.tile([128, 128], BF16)
                    nc.scalar.activation(hT[:], ps2[:],
                                         mybir.ActivationFunctionType.Relu,
                                         bias=bv[:])
                    pst = pstr.tile([128, 128], BF16)
                    nc.tensor.transpose(pst[:], hT[:], ident[:])
                    nc.vector.tensor_copy(hstage[:, r, :], pst[:])
                else:
                    nc.scalar.activation(ostage[:, r * 128:(r + 1) * 128], ps2[:],
                                         mybir.ActivationFunctionType.Identity,
                                         bias=bv[:])
        if layer == 1:
            hview = hout.ap().rearrange("(t p) f -> p t f", p=128)
            nc.sync.dma_start(hview, hstage[:])
        else:
            nc.sync.dma_start(tout[:], ostage[:])
    nc.compile()
    return nc


def _wrap_idxs(stream, GROUPS_, GN):
    """[GROUPS*GN] idx stream -> [128, GROUPS*GN/16] int16 sbuf wrap layout,
    wrapped independently per gather call (per group)."""
    a = stream.reshape(GROUPS_, GN // 16, 16).transpose(0, 2, 1)  # [G,16,C]
    a = a.reshape(GROUPS_ * 16, GN // 16)
    a = np.concatenate([a[g * 16:(g + 1) * 16] for g in range(GROUPS_)], axis=1)
    return np.tile(a, (8, 1)).astype(np.int16)


def preprocess(x, edge_index):
    src = np.asarray(edge_index[0], dtype=np.int64)
    dst = np.asarray(edge_index[1], dtype=np.int64)
    E = src.shape[0]
    deg = np.bincount(dst, minlength=N)
    recip = (1.0 / np.maximum(deg, 1)).astype(np.float32)

    core = dst // NPC
    rloc = (dst % NPC) // 128
    slot = (dst % NPC) % 128
    trow = (src // NPC) * NPAD + (src % NPC)
    half = (trow >= HALF_ROWS).astype(np.int64)
    loc = trow - half * HALF_ROWS

    cell = (core * RANGES + rloc) * 2 + half          # 0 .. 640
    order = np.argsort(cell, kind="stable")
    cs = cell[order]
    counts = np.bincount(cell, minlength=CORES * RANGES * 2)
    BPH = max(10, int(np.ceil(counts.max() / 128)))
    CELL = BPH * 128
    starts = np.zeros_like(counts)
    starts[1:] = np.cumsum(counts)[:-1]
    pos = np.arange(E) - starts[cs]

    core_s = cs // (RANGES * 2)
    rem = cs % (RANGES * 2)
    rloc_s = rem // 2
    half_s = rem % 2

    idxarr = np.zeros((CORES, 2, RANGES * CELL), np.int16)
    idxarr[core_s, half_s, rloc_s * CELL + pos] = loc[order].astype(np.int16)
    BPR = 2 * BPH
    oh = np.zeros((CORES, 128, RANGES * BPR, 128), NP_FP8)
    blk = rloc_s * BPR + half_s * BPH + pos // 128
    oh[core_s, pos % 128, blk, slot[order]] = 1.0

    GN = RPG * CELL
    wrapA = [_wrap_idxs(idxarr[c, 0], GROUPS, GN) for c in range(CORES)]
    wrapB = [_wrap_idxs(idxarr[c, 1], GROUPS, GN) for c in range(CORES)]

    # padded replicated table of x
    xt = np.zeros((TROWS, D), NP_BF16)
    xv = np.asarray(x, dtype=np.float32)
    for c in range(CORES):
        xt[c * NPAD:c * NPAD + NPC] = xv[c * NPC:(c + 1) * NPC].astype(NP_BF16)
    xT = []
    recipb = []
    for c in range(CORES):
        t = np.zeros((128, NPAD), NP_BF16)
        t[:, :NPC] = xv[c * NPC:(c + 1) * NPC].T.astype(NP_BF16)
        xT.append(t)
        rb = np.zeros((NPAD,), np.float32)
        rb[:NPC] = recip[c * NPC:(c + 1) * NPC]
        recipb.append(np.broadcast_to(rb.astype(NP_BF16), (128, NPAD)).copy())
    return BPH, oh, wrapA, wrapB, xt, xT, recipb


def kernel(x, edge_index, W1_l, b1, W1_r, W2_l, b2, W2_r, _timing=None):
    BPH, oh, wrapA, wrapB, xt, xT, recipb = preprocess(x, edge_index)

    key = BPH
    if key not in _prog_cache:
        _prog_cache[key] = (build_program(BPH, 1), build_program(BPH, 2))
    nc1, nc2 = _prog_cache[key]

    def wmat(w):
        return np.asarray(w, dtype=np.float32).astype(NP_BF16)

    def bcol(b):
        return np.asarray(b, dtype=np.float32).reshape(128, 1)

    maps1 = []
    for c in range(CORES):
        maps1.append(dict(table=xt, ohp=oh[c], idxA=wrapA[c], idxB=wrapB[c],
                          xT=xT[c], recipb=recipb[c], Wl=wmat(W1_l),
                          Wr=wmat(W1_r), bvec=bcol(b1),
                          ident=np.eye(128, dtype=NP_BF16)))
    r1 = bass_utils.run_bass_kernel_spmd(nc1, maps1, core_ids=list(range(CORES)))
    ht = np.concatenate([r1.results[c]["hout"] for c in range(CORES)], axis=0)

    maps2 = []
    for c in range(CORES):
        hT_own = np.ascontiguousarray(r1.results[c]["hout"].T)
        maps2.append(dict(table=ht, ohp=oh[c], idxA=wrapA[c], idxB=wrapB[c],
                          xT=hT_own, recipb=recipb[c], Wl=wmat(W2_l),
                          Wr=wmat(W2_r), bvec=bcol(b2)))
    r2 = bass_utils.run_bass_kernel_spmd(nc2, maps2, core_ids=list(range(CORES)))
    if _timing is not None:
        _timing["nc1"] = nc1
        _timing["nc2"] = nc2

    out = np.empty((N, D), np.float32)
    for c in range(CORES):
        out[c * NPC:(c + 1) * NPC] = r2.results[c]["tout"].T[:NPC]
    return out

